# revision 1
# baseline (speedup 1.0000x reference)
"""CTC batch cost (keras ctc_batch_cost port) on 8 Trainium2 NeuronCores.

Strategy (data parallel over batch, 32 rows per core):
  - Stream y_pred tiles [128p=(8 batch x 16 t), 512c] from DRAM.
  - ScalarE: q = 512*(p + 1e-7)  (eps from keras, 512 keeps prob-space DP
    magnitudes ~O(1) per step; corrected at the end by T*log(512)).
  - GPSIMD ap_gather: per 16-partition group (one batch row, 16 timesteps)
    gather the 129 extended-label classes (+15 pad) -> [128, 144].
  - GPSIMD multiply by a valid-state mask (zeroing states beyond
    2*label_len, which otherwise inflate the row max by ~e^40 and push
    the end-state sum below fp32/Ln range); doubles as the fp32->bf16
    cast for the DP datapath.
  - Flatten-DMA into PB[t-window] tiles laid out [32 batch rows, 16*132]
    so each DP step reads a contiguous [32, 129] slice.
  - VectorE (bf16): prob-space CTC forward DP, 255 steps, 4 ops/step:
        a_new[s] = (a[s] + a[s-1] + skip[s]*a[s-2]) * q_t[s]
    with a row-max rescale every 8 steps folded into the next step's
    fused (tensor*scalar)*tensor op; log(max) factors batched into one Ln.
  - Final: masked sum over the two CTC end states; the sum can sit ~e^-50
    below the row max where the HW Ln table is garbage, so Ln of its 4th
    root (two Sqrts) weighted by 4; then one affine correction.

HW pitfalls found on the way (CoreSim is clean for both):
  - ap_gather idxs_ap must start 4-byte aligned or lanes misgather.
  - ACT Ln saturates around ln(1e-19); inputs must stay well above.
"""

import numpy as np

B, T, C, L = 256, 256, 512, 64
NCORES = 8
BPC = B // NCORES  # 32 batch rows per core
S = 2 * L + 1  # 129 extended states
NIDX = 144  # gather index count (multiple of 16; 129 real + 15 pad)
BLK = 132  # per-timestep block width in PB tiles
BLANK = C - 1
EPS = 1e-7
CSCALE = 512.0
RES_EVERY = 8
CONST = float(T * np.log(CSCALE))  # total log correction for the 512 folding

_cache = {}


def _build_program():
    import concourse.bass as bass
    import concourse.tile as tile
    from concourse import bacc, mybir

    f32 = mybir.dt.float32
    bf16 = mybir.dt.bfloat16
    i16 = mybir.dt.int16
    Act = mybir.ActivationFunctionType
    Alu = mybir.AluOpType

    nc = bacc.Bacc("TRN2", debug=False, enable_asserts=False,
                   target_bir_lowering=False)

    y = nc.dram_tensor("y", [BPC, T, C], f32, kind="ExternalInput").ap()
    idxw = nc.dram_tensor("idxw", [4, 128, NIDX // 16], i16,
                          kind="ExternalInput").ap()
    vmw = nc.dram_tensor("vmw", [4, 128, NIDX], bf16,
                         kind="ExternalInput").ap()
    skip = nc.dram_tensor("skip", [BPC, S], bf16, kind="ExternalInput").ap()
    em = nc.dram_tensor("em", [BPC, S], bf16, kind="ExternalInput").ap()
    loss = nc.dram_tensor("loss", [BPC, 1], f32, kind="ExternalOutput").ap()

    with tile.TileContext(nc) as tc:
        with (
            tc.tile_pool(name="pb", bufs=16) as pbp,
            tc.tile_pool(name="yin", bufs=3) as yp,
            tc.tile_pool(name="ysc", bufs=3) as ysp,
            tc.tile_pool(name="gt", bufs=3) as gtp,
            tc.tile_pool(name="small", bufs=1) as sp,
            tc.tile_pool(name="rp", bufs=2) as rp,
        ):
            # --- constants / indices ---
            # one tile per batch-group: ap_gather's idxs_ap must be
            # 4-byte aligned on HW (tile bases are; int16 slices at odd
            # element offsets are not and silently misgather)
            idx_ts = []
            vm_ts = []
            for bg in range(4):
                idx_ts.append(sp.tile([128, NIDX // 16], i16,
                                      tag=f"idx{bg}", name=f"idx{bg}"))
                nc.sync.dma_start(idx_ts[bg][:, :], idxw[bg])
                # valid-state mask: zero states beyond 2*label_len so fake
                # longer-label paths can't inflate the row max (keeps the
                # end-state sum within fp32/Ln range)
                vm_ts.append(sp.tile([128, NIDX], bf16,
                                     tag=f"vm{bg}", name=f"vm{bg}"))
                nc.sync.dma_start(vm_ts[bg][:, :], vmw[bg])
            skip_t = sp.tile([BPC, S], bf16, tag="skip")
            nc.sync.dma_start(skip_t[:, :], skip)
            em_t = sp.tile([BPC, S], bf16, tag="em")
            nc.sync.dma_start(em_t[:, :], em)

            # --- gather phase: 16 t-windows x 4 batch-groups ---
            pb = []
            for tw in range(16):
                pb.append(pbp.tile([BPC, 16 * BLK], bf16, tag="pb", name=f"pb{tw}"))
            for tw in range(16):
                for bg in range(4):
                    yt = yp.tile([128, C], f32, tag="y", name=f"yt_{tw}_{bg}")
                    nc.sync.dma_start(
                        yt[:, :],
                        y[8 * bg:8 * bg + 8, 16 * tw:16 * tw + 16, :],
                    )
                    ys = ysp.tile([128, C], f32, tag="ys", name=f"ys_{tw}_{bg}")
                    nc.scalar.activation(ys[:, :], yt[:, :], Act.Copy,
                                         bias=CSCALE * EPS, scale=CSCALE)
                    gt = gtp.tile([128, NIDX], f32, tag="gt", name=f"gt_{tw}_{bg}")
                    nc.gpsimd.ap_gather(
                        gt[:, :], ys[:, :], idx_ts[bg][:, :],
                        channels=128, num_elems=C, d=1, num_idxs=NIDX,
                    )
                    # vmask multiply doubles as the fp32 -> bf16 cast
                    gt2 = gtp.tile([128, NIDX], bf16, tag="gt2",
                                   name=f"gt2_{tw}_{bg}")
                    nc.gpsimd.tensor_mul(gt2[:, :], gt[:, :], vm_ts[bg][:, :])
                    nc.sync.dma_start(
                        pb[tw][8 * bg:8 * bg + 8, :].rearrange(
                            "p (q s) -> p q s", q=16),
                        gt2[:, 0:BLK],
                    )

            # --- DP phase on VectorE ---
            # aw columns: 0,1 guard zeros; col j+2 = state j (j in 0..128)
            aw0 = sp.tile([BPC, S + 2], bf16, tag="aw0")
            aw1 = sp.tile([BPC, S + 2], bf16, tag="aw1")
            tmp1 = sp.tile([BPC, S], bf16, tag="tmp1")
            tmp2 = sp.tile([BPC, S], bf16, tag="tmp2")
            mlog = sp.tile([BPC, 32], f32, tag="mlog")
            ln_t = sp.tile([BPC, 32], f32, tag="ln")
            acc_t = sp.tile([BPC, 1], f32, tag="acc")
            loss_t = sp.tile([BPC, 1], f32, tag="loss")

            nc.vector.memset(aw0[:, :], 0.0)
            nc.vector.memset(aw1[:, :], 0.0)
            # ln(1)=0 filler so unused mlog cols contribute nothing
            nc.vector.memset(mlog[:, :], 1.0)

            # init: alpha0 over states 0,1 = q_0 at those states
            nc.vector.tensor_copy(aw0[:, 2:4], pb[0][:, 0:2])

            cur, nxt = aw0, aw1
            pending_r = None
            k = 0
            for t in range(1, T):
                tw, qq = divmod(t, 16)
                qt = pb[tw][:, qq * BLK:qq * BLK + S]
                # tmp1 = a[s] + a[s-1]
                nc.vector.tensor_add(tmp1[:, :], cur[:, 2:2 + S],
                                     cur[:, 1:1 + S])
                # tmp2 = skip[s] * a[s-2]
                nc.vector.tensor_mul(tmp2[:, :], cur[:, 0:S], skip_t[:, :])
                nc.vector.tensor_add(tmp1[:, :], tmp1[:, :], tmp2[:, :])
                if pending_r is None:
                    nc.vector.tensor_mul(nxt[:, 2:2 + S], tmp1[:, :], qt)
                else:
                    # fold the previous epoch's 1/max rescale into the mul
                    nc.vector.scalar_tensor_tensor(
                        nxt[:, 2:2 + S], tmp1[:, :], pending_r, qt,
                        op0=Alu.mult, op1=Alu.mult)
                    pending_r = None
                if t % RES_EVERY == RES_EVERY - 1 and t != T - 1:
                    nc.vector.reduce_max(mlog[:, k:k + 1], nxt[:, 2:2 + S],
                                         axis=mybir.AxisListType.X)
                    r_t = rp.tile([BPC, 1], f32, tag="r", name=f"r_{t}")
                    nc.vector.reciprocal(r_t[:, :], mlog[:, k:k + 1])
                    pending_r = r_t
                    k += 1
                cur, nxt = nxt, cur

            # final: masked end-state sum into mlog col 31, batch all Ln's.
            # The sum can be ~e^-50 below 1 where HW Ln is garbage, so take
            # Ln of its 4th root (accurate down to ~1e-35) and weight by 4.
            nc.vector.scalar_tensor_tensor(
                tmp2[:, :], cur[:, 2:2 + S], 1.0, em_t[:, :],
                op0=Alu.mult, op1=Alu.mult, accum_out=mlog[:, 31:32],
            )
            nc.scalar.activation(mlog[:, 31:32], mlog[:, 31:32], Act.Sqrt)
            nc.scalar.activation(mlog[:, 31:32], mlog[:, 31:32], Act.Sqrt)
            nc.scalar.activation(ln_t[:, :], mlog[:, :], Act.Ln)
            nc.vector.reduce_sum(acc_t[:, :], ln_t[:, 0:31],
                                 axis=mybir.AxisListType.X)
            nc.vector.scalar_tensor_tensor(
                acc_t[:, :], ln_t[:, 31:32], 4.0, acc_t[:, :],
                op0=Alu.mult, op1=Alu.add,
            )
            # loss = -(sum of logs) + T*log(512)
            nc.scalar.activation(loss_t[:, :], acc_t[:, :], Act.Copy,
                                 bias=CONST, scale=-1.0)
            nc.sync.dma_start(loss, loss_t[:, :])

    nc.compile()
    return nc


def _host_prep(y_true, y_pred):
    """Build per-core input maps from full inputs."""
    y_pred = np.ascontiguousarray(np.asarray(y_pred, dtype=np.float32))
    y_true = np.asarray(y_true)
    labels = y_true[:, :L].astype(np.int64)
    lab_len = y_true[:, L].astype(np.int64)

    extpad = np.full((B, NIDX), BLANK, dtype=np.int16)
    extpad[:, 1:2 * L:2] = labels.astype(np.int16)

    import ml_dtypes
    bf = ml_dtypes.bfloat16
    skip = np.zeros((B, S), dtype=bf)
    skip[:, 3:S:2] = (labels[:, 1:] != labels[:, :-1]).astype(bf)

    em = np.zeros((B, S), dtype=bf)
    rows = np.arange(B)
    em[rows, 2 * lab_len] = 1.0
    em[rows, 2 * lab_len - 1] = 1.0

    i = np.arange(NIDX)
    svals = np.arange(NIDX)
    in_maps = []
    for c in range(NCORES):
        b0 = BPC * c
        idxw = np.zeros((4, 128, NIDX // 16), dtype=np.int16)
        vmw = np.zeros((4, 128, NIDX), dtype=bf)
        for bg in range(4):
            for g in range(8):
                b = b0 + 8 * bg + g
                idxw[bg, 16 * g + i % 16, i // 16] = extpad[b, i]
                vmw[bg, 16 * g:16 * g + 16, :] = (
                    svals <= 2 * lab_len[b]).astype(bf)[None, :]
        in_maps.append({
            "y": y_pred[b0:b0 + BPC],
            "idxw": idxw,
            "vmw": vmw,
            "skip": skip[b0:b0 + BPC],
            "em": em[b0:b0 + BPC],
        })
    return in_maps


def _run(in_maps, trace=False):
    from concourse.bass_utils import run_bass_kernel_spmd

    if "nc" not in _cache:
        _cache["nc"] = _build_program()
    return run_bass_kernel_spmd(
        _cache["nc"], in_maps, core_ids=list(range(NCORES)), trace=trace,
    )


def kernel(y_true, y_pred):
    in_maps = _host_prep(y_true, y_pred)
    res = _run(in_maps)
    return np.concatenate([r["loss"] for r in res.results], axis=0)



# revision 4
# speedup vs baseline: 1.3497x; 1.3497x over previous
"""CTC batch cost (keras ctc_batch_cost port) on 8 Trainium2 NeuronCores.

Strategy (data parallel over batch, 32 rows per core), v2:
  - The serial CTC scan is split at the midpoint into a forward alpha
    chain (t=0..127) and a backward gamma chain (t=255..128).  The
    backward chain is stored STATE-REVERSED, which turns its transposed
    recurrence into the exact same shifted-add form as the forward one:
        x'[s] = (x[s] + x[s-1] + m[s]*x[s-2]) * q[s]
    Both chains are stacked on partitions (0..31 fwd rows, 32..63 bwd
    rows) so one [64,129] DVE op advances both -> half the serial steps
    of a single 255-step scan at identical per-op cost.
  - Host ships y with the second time-half reversed (yv[:,128+j] =
    y[:,255-j]) so both chains consume ascending 16-step windows; the
    backward gather indices are state-reversed host data.
  - Gather path per (window, row-group): DMA y tile [128p=(8 rows x
    16 t), 516] fp32 with 4 pre-zeroed pad cols; GPSIMD ap_gather of the
    129 extended-label classes (invalid states index the zero column,
    masking fake paths); one ACT op applies keras' eps + a 512x scale
    (keeps prob-space DP ~O(1)) and casts to bf16; flatten-DMA into
    PB[w] tiles [64, 16*132] so each DP step reads one [64,129] slice.
  - Rescale: row max every 12 steps, folded into the next step's
    (tensor*scalar)*tensor op; log(max) factors batched into one Ln.
  - Final: one more maskless A-step on the bwd side gives beta_127
    (reversed); DMA to partitions 0..31, gather-reverse, then a dot with
    alpha_127 via accum_out.  The dot can sit far below 1 where the HW
    Ln table is garbage, so Ln of its 4th root (two Sqrts) weighted 4.

HW pitfalls (from the v1 baseline; CoreSim clean for both):
  - ap_gather idxs_ap must start 4-byte aligned or lanes misgather.
  - ap_gather requires d*dtype_size % 4 == 0 (hence fp32 gathers).
  - ACT Ln saturates around ln(1e-19); inputs must stay well above.
"""

import numpy as np

B, T, C, L = 256, 256, 512, 64
NCORES = 8
BPC = B // NCORES  # 32 batch rows per core
S = 2 * L + 1  # 129 extended states
NIDX = 144  # gather index count (multiple of 16; 129 real + 15 pad)
BLK = 132  # per-timestep block width in PB tiles
YW = 516  # y tile width: 512 classes + 4 zero pad cols (col 512 = mask)
BLANK = C - 1
EPS = 1e-7
CSCALE = 512.0
RES_EVERY = 12
HALF = T // 2  # 128 double-steps
CONST = float(T * np.log(CSCALE))  # total log correction for the 512 folding

_cache = {}


def _build_program():
    import concourse.bass as bass
    import concourse.tile as tile
    from concourse import bacc, mybir

    f32 = mybir.dt.float32
    bf16 = mybir.dt.bfloat16
    i16 = mybir.dt.int16
    Act = mybir.ActivationFunctionType
    Alu = mybir.AluOpType

    nc = bacc.Bacc("TRN2", debug=False, enable_asserts=False,
                   target_bir_lowering=False)

    yv = nc.dram_tensor("yv", [BPC, T, C], f32, kind="ExternalInput").ap()
    idxw = nc.dram_tensor("idxw", [8, 128, NIDX // 16], i16,
                          kind="ExternalInput").ap()
    rvw = nc.dram_tensor("rvw", [32, NIDX // 16], i16,
                         kind="ExternalInput").ap()
    km = nc.dram_tensor("km", [2 * BPC, S], bf16, kind="ExternalInput").ap()
    emr = nc.dram_tensor("emr", [2 * BPC, S], bf16, kind="ExternalInput").ap()
    loss = nc.dram_tensor("loss", [BPC, 1], f32, kind="ExternalOutput").ap()

    P2 = 2 * BPC  # 64 partitions: fwd rows + bwd rows

    with tile.TileContext(nc) as tc:
        with (
            tc.tile_pool(name="pb", bufs=8) as pbp,
            tc.tile_pool(name="yin", bufs=1) as yp,
            tc.tile_pool(name="gt", bufs=3) as gtp,
            tc.tile_pool(name="ga", bufs=3) as gap,
            tc.tile_pool(name="small", bufs=1) as sp,
            tc.tile_pool(name="rp", bufs=2) as rp,
        ):
            # --- constants / indices ---
            # one idx tile per (bg, half): ap_gather's idxs_ap must be
            # 4-byte aligned on HW (tile bases are)
            idx_ts = []
            for j in range(8):
                it = sp.tile([128, NIDX // 16], i16, tag=f"idx{j}",
                             name=f"idx{j}")
                nc.sync.dma_start(it[:, :], idxw[j])
                idx_ts.append(it)
            rv_t = sp.tile([32, NIDX // 16], i16, tag="rv", name="rv_t")
            nc.sync.dma_start(rv_t[:, :], rvw)
            km_t = sp.tile([P2, S], bf16, tag="km", name="km_t")
            nc.sync.dma_start(km_t[:, :], km)
            emr_t = sp.tile([P2, S], bf16, tag="emr", name="emr_t")
            nc.sync.dma_start(emr_t[:, :], emr)

            # 3 rotating y tiles with pre-zeroed pad cols (the gather's
            # zero column for invalid-state masking)
            yts = []
            for j in range(3):
                yt = yp.tile([128, YW], f32, tag=f"y{j}", name=f"yt{j}")
                nc.vector.memset(yt[:, C:YW], 0.0)
                yts.append(yt)

            pb = []
            for w in range(8):
                pb.append(pbp.tile([P2, 16 * BLK], bf16, tag="pb",
                                   name=f"pb{w}"))

            # --- gather phase: window pairs (w fwd, w+8 bwd rev) ---
            ui = 0
            for w in range(8):
                for v in (w, w + 8):
                    half = 0 if v < 8 else 1
                    pbase = 0 if half == 0 else BPC
                    for bg in range(4):
                        yt = yts[ui % 3]
                        ui += 1
                        nc.sync.dma_start(
                            yt[:, 0:C],
                            yv[8 * bg:8 * bg + 8, 16 * v:16 * v + 16, :],
                        )
                        gt = gtp.tile([128, NIDX], f32, tag="gt",
                                      name=f"gt_{v}_{bg}")
                        nc.gpsimd.ap_gather(
                            gt[:, :], yt[:, :], idx_ts[2 * bg + half][:, :],
                            channels=128, num_elems=YW, d=1, num_idxs=NIDX,
                        )
                        # eps + 512x scale + fp32 -> bf16 cast in one ACT op
                        ga = gap.tile([128, NIDX], bf16, tag="ga",
                                      name=f"ga_{v}_{bg}")
                        nc.scalar.activation(ga[:, :], gt[:, :], Act.Copy,
                                             bias=CSCALE * EPS, scale=CSCALE)
                        nc.sync.dma_start(
                            pb[w][pbase + 8 * bg:pbase + 8 * bg + 8,
                                  :].rearrange("p (q s) -> p q s", q=16),
                            ga[:, 0:BLK],
                        )

            # --- DP phase on VectorE: 127 stacked double-steps ---
            # aw columns: 0,1 guard zeros; col j+2 = state j (j in 0..128)
            aw0 = sp.tile([P2, S + 2], bf16, tag="aw0", name="aw0")
            aw1 = sp.tile([P2, S + 2], bf16, tag="aw1", name="aw1")
            t1 = sp.tile([P2, S], bf16, tag="t1", name="t1")
            t2 = sp.tile([P2, S], bf16, tag="t2", name="t2")
            mlog = sp.tile([P2, 32], f32, tag="mlog", name="mlog")
            ln_t = sp.tile([P2, 32], f32, tag="ln", name="ln_t")
            acc_t = sp.tile([P2, 1], f32, tag="acc", name="acc_t")
            accb = sp.tile([BPC, 1], f32, tag="accb", name="accb")
            bstage = sp.tile([P2, S], f32, tag="bstage", name="bstage")
            bmov = sp.tile([BPC, S + 3], f32, tag="bmov", name="bmov")
            brev = sp.tile([BPC, NIDX], f32, tag="brev", name="brev")
            loss_t = sp.tile([BPC, 1], f32, tag="loss", name="loss_t")

            nc.vector.memset(aw0[:, :], 0.0)
            nc.vector.memset(aw1[:, :], 0.0)
            nc.vector.memset(bmov[:, :], 0.0)
            # ln(1)=0 filler so unused mlog cols contribute nothing
            nc.vector.memset(mlog[:, :], 1.0)

            # init: fwd alpha0 = q_0 at states 0,1; bwd W = q_255*em rev
            nc.vector.tensor_mul(aw0[:, 2:2 + S], pb[0][:, 0:S], emr_t[:, :])
            nc.vector.tensor_copy(aw0[0:BPC, 2:4], pb[0][0:BPC, 0:2])

            cur, nxt = aw0, aw1
            pending_r = None
            e = 0
            for i in range(1, HALF):
                w, tl = divmod(i, 16)
                qt = pb[w][:, tl * BLK:tl * BLK + S]
                nc.vector.tensor_add(t1[:, :], cur[:, 2:2 + S],
                                     cur[:, 1:1 + S])
                nc.vector.tensor_mul(t2[:, :], cur[:, 0:S], km_t[:, :])
                nc.vector.tensor_add(t1[:, :], t1[:, :], t2[:, :])
                if pending_r is None:
                    nc.vector.tensor_mul(nxt[:, 2:2 + S], t1[:, :], qt)
                else:
                    # fold the previous epoch's 1/max rescale into the mul
                    nc.vector.scalar_tensor_tensor(
                        nxt[:, 2:2 + S], t1[:, :], pending_r, qt,
                        op0=Alu.mult, op1=Alu.mult)
                    pending_r = None
                if i % RES_EVERY == RES_EVERY - 1 and i != HALF - 1:
                    nc.vector.reduce_max(mlog[:, e:e + 1], nxt[:, 2:2 + S],
                                         axis=mybir.AxisListType.X)
                    r_t = rp.tile([P2, 1], f32, tag="r", name=f"r_{i}")
                    nc.vector.reciprocal(r_t[:, :], mlog[:, e:e + 1])
                    pending_r = r_t
                    e += 1
                cur, nxt = nxt, cur

            # --- final combine ---
            # one more maskless A-step on the bwd half: beta_127 reversed
            nc.vector.tensor_add(t1[BPC:P2, :], cur[BPC:P2, 2:2 + S],
                                 cur[BPC:P2, 1:1 + S])
            nc.vector.tensor_mul(t2[BPC:P2, :], cur[BPC:P2, 0:S],
                                 km_t[BPC:P2, :])
            # write as fp32 (gather needs 4-byte dtype for the reversal),
            # then move to partitions 0..31 next to alpha
            nc.vector.tensor_add(bstage[BPC:P2, :], t1[BPC:P2, :],
                                 t2[BPC:P2, :])
            nc.sync.dma_start(bmov[:, 0:S], bstage[BPC:P2, :])
            nc.gpsimd.ap_gather(
                brev[:, :], bmov[:, :], rv_t[:, :],
                channels=32, num_elems=S + 3, d=1, num_idxs=NIDX,
            )
            # cast alpha to fp32 then dot with reversed beta, accum to D
            alpha32 = sp.tile([BPC, S], f32, tag="al32", name="alpha32")
            nc.vector.tensor_copy(alpha32[:, :], cur[0:BPC, 2:2 + S])
            nc.vector.scalar_tensor_tensor(
                bstage[0:BPC, :], alpha32[:, :], 1.0, brev[:, 0:S],
                op0=Alu.mult, op1=Alu.mult, accum_out=mlog[0:BPC, 31:32],
            )
            # D can sit far below 1 where HW Ln is garbage: Ln of its 4th
            # root (two Sqrts), weighted by 4 in the final sum.
            nc.scalar.activation(mlog[0:BPC, 31:32], mlog[0:BPC, 31:32],
                                 Act.Sqrt)
            nc.scalar.activation(mlog[0:BPC, 31:32], mlog[0:BPC, 31:32],
                                 Act.Sqrt)
            nc.scalar.activation(ln_t[:, :], mlog[:, :], Act.Ln)
            nc.vector.reduce_sum(acc_t[:, :], ln_t[:, 0:31],
                                 axis=mybir.AxisListType.X)
            nc.vector.scalar_tensor_tensor(
                acc_t[:, :], ln_t[:, 31:32], 4.0, acc_t[:, :],
                op0=Alu.mult, op1=Alu.add,
            )
            # fold bwd-partition log sums onto the fwd partitions
            nc.sync.dma_start(accb[:, :], acc_t[BPC:P2, :])
            nc.vector.tensor_add(acc_t[0:BPC, :], acc_t[0:BPC, :],
                                 accb[:, :])
            # loss = -(sum of logs) + T*log(512)
            nc.scalar.activation(loss_t[:, :], acc_t[0:BPC, :], Act.Copy,
                                 bias=CONST, scale=-1.0)
            nc.sync.dma_start(loss, loss_t[:, :])

    nc.compile()
    return nc


def _host_prep(y_true, y_pred):
    """Build per-core input maps from full inputs."""
    import ml_dtypes

    bf = ml_dtypes.bfloat16
    y_pred = np.asarray(y_pred, dtype=np.float32)
    y_true = np.asarray(y_true)
    labels = y_true[:, :L].astype(np.int64)
    lab_len = y_true[:, L].astype(np.int64)

    # y with the second time-half reversed: yv[:,128+j] = y[:,255-j]
    yv = np.concatenate([y_pred[:, :HALF], y_pred[:, T - 1:HALF - 1:-1]],
                        axis=1)
    yv = np.ascontiguousarray(yv)

    # extended labels with invalid states (s > 2*len) pointing at the
    # zero column (C); gather positions >= S also go to the zero column
    ext = np.full((B, NIDX), C, dtype=np.int64)
    ext[:, 0:S:2] = BLANK
    ext[:, 1:S:2] = labels
    svals = np.arange(NIDX)
    ext[svals[None, :] > (2 * lab_len)[:, None]] = C
    extr = np.full((B, NIDX), C, dtype=np.int64)
    extr[:, 0:S] = ext[:, S - 1::-1]  # state-reversed for the bwd half

    # skip masks: fwd k[s]=1 at odd s with distinct labels; bwd mirrored
    k = np.zeros((B, S), dtype=np.float32)
    k[:, 3:S:2] = (labels[:, 1:] != labels[:, :-1]).astype(np.float32)
    kL = np.zeros((B, S), dtype=np.float32)
    kL[:, :S - 2] = k[:, 2:]
    kmr = kL[:, ::-1]

    # end-state mask, reversed (bwd init: W = q_255 * em_rev)
    em = np.zeros((B, S), dtype=np.float32)
    rows = np.arange(B)
    em[rows, 2 * lab_len] = 1.0
    em[rows, 2 * lab_len - 1] = 1.0
    emrev = em[:, ::-1]

    # beta-reversal indices for the final dot (shared by all cores):
    # wrapped 16-partition layout, same for both 16-row groups
    i = np.arange(NIDX)
    rvals = np.where(i < S, S - 1 - i, S + 1).astype(np.int16)
    rvw = np.zeros((32, NIDX // 16), dtype=np.int16)
    for g in range(2):
        rvw[16 * g + i % 16, i // 16] = rvals

    in_maps = []
    for c in range(NCORES):
        b0 = BPC * c
        idxw = np.zeros((8, 128, NIDX // 16), dtype=np.int16)
        for bg in range(4):
            for g in range(8):
                b = b0 + 8 * bg + g
                idxw[2 * bg + 0, 16 * g + i % 16, i // 16] = ext[b, i]
                idxw[2 * bg + 1, 16 * g + i % 16, i // 16] = extr[b, i]
        kmc = np.concatenate([k[b0:b0 + BPC], kmr[b0:b0 + BPC]],
                             axis=0).astype(bf)
        emc = np.concatenate([np.zeros((BPC, S), np.float32),
                              emrev[b0:b0 + BPC]], axis=0).astype(bf)
        in_maps.append({
            "yv": yv[b0:b0 + BPC],
            "idxw": idxw,
            "rvw": rvw,
            "km": kmc,
            "emr": emc,
        })
    return in_maps


def _run(in_maps, trace=False):
    from concourse.bass_utils import run_bass_kernel_spmd

    if "nc" not in _cache:
        _cache["nc"] = _build_program()
    return run_bass_kernel_spmd(
        _cache["nc"], in_maps, core_ids=list(range(NCORES)), trace=trace,
    )


def kernel(y_true, y_pred):
    in_maps = _host_prep(y_true, y_pred)
    res = _run(in_maps)
    return np.concatenate([r["loss"] for r in res.results], axis=0)


# revision 14
# speedup vs baseline: 1.7278x; 1.2802x over previous
"""CTC batch cost (keras ctc_batch_cost port) on 8 Trainium2 NeuronCores.

Strategy (data parallel over batch, 32 rows per core), v2:
  - The serial CTC scan is split at the midpoint into a forward alpha
    chain (t=0..127) and a backward gamma chain (t=255..128).  The
    backward chain is stored STATE-REVERSED, which turns its transposed
    recurrence into the exact same shifted-add form as the forward one:
        x'[s] = (x[s] + x[s-1] + m[s]*x[s-2]) * q[s]
    Both chains are stacked on partitions (0..31 fwd rows, 32..63 bwd
    rows) so one [64,129] DVE op advances both -> half the serial steps
    of a single 255-step scan at identical per-op cost.
  - Host ships y with the second time-half reversed (yv[:,128+j] =
    y[:,255-j]) so both chains consume ascending 16-step windows; the
    backward gather indices are state-reversed host data.
  - Gather path per (window, row-group): DMA y tile [128p=(8 rows x
    16 t), 516] fp32 with 4 pre-zeroed pad cols; GPSIMD ap_gather of the
    129 extended-label classes (invalid states index the zero column,
    masking fake paths); one ACT op applies keras' eps + a 512x scale
    (keeps prob-space DP ~O(1)) and casts to bf16; flatten-DMA into
    PB[w] tiles [64, 16*132] so each DP step reads one [64,129] slice.
  - Rescale: row max every 12 steps, folded into the next step's
    (tensor*scalar)*tensor op; log(max) factors batched into one Ln.
  - Final: one more maskless A-step on the bwd side gives beta_127
    (reversed); DMA to partitions 0..31, gather-reverse, then a dot with
    alpha_127 via accum_out.  The dot can sit far below 1 where the HW
    Ln table is garbage, so Ln of its 4th root (two Sqrts) weighted 4.

HW pitfalls (from the v1 baseline; CoreSim clean for both):
  - ap_gather idxs_ap must start 4-byte aligned or lanes misgather.
  - ap_gather requires d*dtype_size % 4 == 0 (hence fp32 gathers).
  - ACT Ln saturates around ln(1e-19); inputs must stay well above.
"""

import numpy as np

B, T, C, L = 256, 256, 512, 64
NCORES = 8
BPC = B // NCORES  # 32 batch rows per core
S = 2 * L + 1  # 129 extended states
NIDX = 144  # gather index count (multiple of 16; 129 real + 15 pad)
BLK = NIDX  # per-timestep block width in PB tiles (= NIDX so the
# per-window flatten-DMA balances to <=3 AP dims)
YW = 516  # y tile width: 512 classes + 4 zero pad cols (col 512 = mask)
BLANK = C - 1
EPS = 1e-7
CSCALE = 512.0
RES_EVERY = 12
HALF = T // 2  # 128 double-steps
CONST = float(T * np.log(CSCALE))  # total log correction for the 512 folding

_cache = {}


def _build_program():
    import concourse.bass as bass
    import concourse.tile as tile
    from concourse import bacc, mybir

    f32 = mybir.dt.float32
    bf16 = mybir.dt.bfloat16
    i16 = mybir.dt.int16
    Act = mybir.ActivationFunctionType
    Alu = mybir.AluOpType

    nc = bacc.Bacc("TRN2", debug=False, enable_asserts=False,
                   target_bir_lowering=False)

    yv = nc.dram_tensor("yv", [BPC, T, C], f32, kind="ExternalInput").ap()
    # 8 idx slots (bg, half) padded to 12 cols so each slot is 4B aligned
    idxw = nc.dram_tensor("idxw", [128, 8 * 12], i16,
                          kind="ExternalInput").ap()
    rvw = nc.dram_tensor("rvw", [32, NIDX // 16], i16,
                         kind="ExternalInput").ap()
    km = nc.dram_tensor("km", [2 * BPC, S], bf16, kind="ExternalInput").ap()
    emr = nc.dram_tensor("emr", [2 * BPC, S], bf16, kind="ExternalInput").ap()
    loss = nc.dram_tensor("loss", [BPC, 1], f32, kind="ExternalOutput").ap()

    P2 = 2 * BPC  # 64 partitions: fwd rows + bwd rows

    with tile.TileContext(nc) as tc:
        with (
            tc.tile_pool(name="pb", bufs=8) as pbp,
            tc.tile_pool(name="yin", bufs=1) as yp,
            tc.tile_pool(name="gt", bufs=6) as gtp,
            tc.tile_pool(name="ga", bufs=3) as gap,
            tc.tile_pool(name="small", bufs=1) as sp,
            tc.tile_pool(name="rp", bufs=2) as rp,
        ):
            # --- constants / indices ---
            # all 8 (bg, half) idx slots in one tile; 12-col slots keep
            # each ap_gather idxs_ap 4-byte aligned (HW requirement)
            idx_t = sp.tile([128, 8 * 12], i16, tag="idx", name="idx_t")
            nc.sync.dma_start(idx_t[:, :], idxw)
            rv_t = sp.tile([32, NIDX // 16], i16, tag="rv", name="rv_t")
            nc.sync.dma_start(rv_t[:, :], rvw)
            km_t = sp.tile([P2, S], bf16, tag="km", name="km_t")
            nc.sync.dma_start(km_t[:, :], km)
            emr_t = sp.tile([P2, S], bf16, tag="emr", name="emr_t")
            nc.sync.dma_start(emr_t[:, :], emr)

            # 3 rotating y tiles with pre-zeroed pad cols (the gather's
            # zero column for invalid-state masking)
            yts = []
            for j in range(3):
                yt = yp.tile([128, YW], f32, tag=f"y{j}", name=f"yt{j}")
                nc.vector.memset(yt[:, C:YW], 0.0)
                yts.append(yt)

            pb = []
            for w in range(8):
                pb.append(pbp.tile([P2, 16 * BLK], bf16, tag="pb",
                                   name=f"pb{w}"))

            # --- gather phase: window pairs (w fwd, w+8 bwd rev) ---
            # y DMAs ride the SP queue; the per-window flatten-DMAs ride
            # the ACT queue so a y DMA blocked on buffer rotation can't
            # head-of-line-block finished windows' pb writes.
            ui = 0
            for w in range(8):
                for v in (w, w + 8):
                    half = 0 if v < 8 else 1
                    pbase = 0 if half == 0 else BPC
                    gab = gtp.tile([128, 4 * NIDX], f32, tag="gab",
                                   name=f"gab_{v}")
                    for bg in range(4):
                        yt = yts[ui % 3]
                        ui += 1
                        nc.sync.dma_start(
                            yt[:, 0:C],
                            yv[8 * bg:8 * bg + 8, 16 * v:16 * v + 16, :],
                        )
                        nc.gpsimd.ap_gather(
                            gab[:, NIDX * bg:NIDX * (bg + 1)],
                            yt[:, :],
                            idx_t[:, 12 * (2 * bg + half):
                                  12 * (2 * bg + half) + NIDX // 16],
                            channels=128, num_elems=YW, d=1, num_idxs=NIDX,
                        )
                    # eps + 512x scale + fp32 -> bf16 cast in one ACT op
                    ga = gap.tile([128, 4 * NIDX], bf16, tag="ga",
                                  name=f"ga_{v}")
                    nc.scalar.activation(ga[:, :], gab[:, :], Act.Copy,
                                         bias=CSCALE * EPS, scale=CSCALE)
                    # flatten-DMAs, split between the ACT queue (HWDGE)
                    # and the Pool queue (SWDGE) to keep each descriptor
                    # generator under the DP critical path
                    for bg in range(4):
                        dst = pb[w][pbase + 8 * bg:pbase + 8 * bg + 8,
                                    :].rearrange("p (q s) -> p q s", q=16)
                        src = ga[:, NIDX * bg:NIDX * (bg + 1)]
                        if bg < 2:
                            nc.scalar.dma_start(dst, src)
                        else:
                            nc.gpsimd.dma_start(dst, src)

            # --- DP phase on VectorE: 127 stacked double-steps ---
            # aw columns: 0,1 guard zeros; col j+2 = state j (j in 0..128)
            aw0 = sp.tile([P2, S + 2], bf16, tag="aw0", name="aw0")
            aw1 = sp.tile([P2, S + 2], bf16, tag="aw1", name="aw1")
            t1 = sp.tile([P2, S], bf16, tag="t1", name="t1")
            t2 = sp.tile([P2, S], bf16, tag="t2", name="t2")
            mlog = sp.tile([P2, 32], f32, tag="mlog", name="mlog")
            ln_t = sp.tile([P2, 32], f32, tag="ln", name="ln_t")
            acc_t = sp.tile([P2, 1], f32, tag="acc", name="acc_t")
            accb = sp.tile([BPC, 1], f32, tag="accb", name="accb")
            bstage = sp.tile([P2, S], f32, tag="bstage", name="bstage")
            bmov = sp.tile([BPC, S + 3], f32, tag="bmov", name="bmov")
            brev = sp.tile([BPC, NIDX], f32, tag="brev", name="brev")
            loss_t = sp.tile([BPC, 1], f32, tag="loss", name="loss_t")

            nc.vector.memset(aw0[:, :], 0.0)
            nc.vector.memset(aw1[:, :], 0.0)
            nc.vector.memset(bmov[:, :], 0.0)
            # ln(1)=0 filler so unused mlog cols contribute nothing
            nc.vector.memset(mlog[:, :], 1.0)

            # init: fwd alpha0 = q_0 at states 0,1; bwd W = q_255*em rev
            nc.vector.tensor_mul(aw0[:, 2:2 + S], pb[0][:, 0:S], emr_t[:, :])
            nc.vector.tensor_copy(aw0[0:BPC, 2:4], pb[0][0:BPC, 0:2])

            cur, nxt = aw0, aw1
            pending_r = None
            e = 0
            for i in range(1, HALF):
                w, tl = divmod(i, 16)
                qt = pb[w][:, tl * BLK:tl * BLK + S]
                nc.vector.tensor_add(t1[:, :], cur[:, 2:2 + S],
                                     cur[:, 1:1 + S])
                nc.vector.tensor_mul(t2[:, :], cur[:, 0:S], km_t[:, :])
                nc.vector.tensor_add(t1[:, :], t1[:, :], t2[:, :])
                if pending_r is None:
                    nc.vector.tensor_mul(nxt[:, 2:2 + S], t1[:, :], qt)
                else:
                    # fold the previous epoch's 1/max rescale into the mul
                    nc.vector.scalar_tensor_tensor(
                        nxt[:, 2:2 + S], t1[:, :], pending_r, qt,
                        op0=Alu.mult, op1=Alu.mult)
                    pending_r = None
                if i % RES_EVERY == RES_EVERY - 1 and i != HALF - 1:
                    nc.vector.reduce_max(mlog[:, e:e + 1], nxt[:, 2:2 + S],
                                         axis=mybir.AxisListType.X)
                    r_t = rp.tile([P2, 1], f32, tag="r", name=f"r_{i}")
                    nc.vector.reciprocal(r_t[:, :], mlog[:, e:e + 1])
                    pending_r = r_t
                    e += 1
                cur, nxt = nxt, cur

            # --- final combine ---
            # one more maskless A-step on the bwd half: beta_127 reversed
            nc.vector.tensor_add(t1[BPC:P2, :], cur[BPC:P2, 2:2 + S],
                                 cur[BPC:P2, 1:1 + S])
            nc.vector.tensor_mul(t2[BPC:P2, :], cur[BPC:P2, 0:S],
                                 km_t[BPC:P2, :])
            # write as fp32 (gather needs 4-byte dtype for the reversal),
            # then move to partitions 0..31 next to alpha
            nc.vector.tensor_add(bstage[BPC:P2, :], t1[BPC:P2, :],
                                 t2[BPC:P2, :])
            nc.sync.dma_start(bmov[:, 0:S], bstage[BPC:P2, :])
            nc.gpsimd.ap_gather(
                brev[:, :], bmov[:, :], rv_t[:, :],
                channels=32, num_elems=S + 3, d=1, num_idxs=NIDX,
            )
            # cast alpha to fp32 then dot with reversed beta, accum to D
            alpha32 = sp.tile([BPC, S], f32, tag="al32", name="alpha32")
            nc.vector.tensor_copy(alpha32[:, :], cur[0:BPC, 2:2 + S])
            nc.vector.scalar_tensor_tensor(
                bstage[0:BPC, :], alpha32[:, :], 1.0, brev[:, 0:S],
                op0=Alu.mult, op1=Alu.mult, accum_out=mlog[0:BPC, 31:32],
            )
            # D can sit far below 1 where HW Ln is garbage: Ln of its 4th
            # root (two Sqrts), weighted by 4 in the final sum.
            nc.scalar.activation(mlog[0:BPC, 31:32], mlog[0:BPC, 31:32],
                                 Act.Sqrt)
            nc.scalar.activation(mlog[0:BPC, 31:32], mlog[0:BPC, 31:32],
                                 Act.Sqrt)
            nc.scalar.activation(ln_t[:, :], mlog[:, :], Act.Ln)
            nc.vector.reduce_sum(acc_t[:, :], ln_t[:, 0:31],
                                 axis=mybir.AxisListType.X)
            nc.vector.scalar_tensor_tensor(
                acc_t[:, :], ln_t[:, 31:32], 4.0, acc_t[:, :],
                op0=Alu.mult, op1=Alu.add,
            )
            # fold bwd-partition log sums onto the fwd partitions
            nc.sync.dma_start(accb[:, :], acc_t[BPC:P2, :])
            nc.vector.tensor_add(acc_t[0:BPC, :], acc_t[0:BPC, :],
                                 accb[:, :])
            # loss = -(sum of logs) + T*log(512)
            nc.scalar.activation(loss_t[:, :], acc_t[0:BPC, :], Act.Copy,
                                 bias=CONST, scale=-1.0)
            nc.sync.dma_start(loss, loss_t[:, :])

    nc.compile()
    return nc


def _host_prep(y_true, y_pred):
    """Build per-core input maps from full inputs."""
    import ml_dtypes

    bf = ml_dtypes.bfloat16
    y_pred = np.asarray(y_pred, dtype=np.float32)
    y_true = np.asarray(y_true)
    labels = y_true[:, :L].astype(np.int64)
    lab_len = y_true[:, L].astype(np.int64)

    # y with the second time-half reversed: yv[:,128+j] = y[:,255-j]
    yv = np.concatenate([y_pred[:, :HALF], y_pred[:, T - 1:HALF - 1:-1]],
                        axis=1)
    yv = np.ascontiguousarray(yv)

    # extended labels with invalid states (s > 2*len) pointing at the
    # zero column (C); gather positions >= S also go to the zero column
    ext = np.full((B, NIDX), C, dtype=np.int64)
    ext[:, 0:S:2] = BLANK
    ext[:, 1:S:2] = labels
    svals = np.arange(NIDX)
    ext[svals[None, :] > (2 * lab_len)[:, None]] = C
    extr = np.full((B, NIDX), C, dtype=np.int64)
    extr[:, 0:S] = ext[:, S - 1::-1]  # state-reversed for the bwd half

    # skip masks: fwd k[s]=1 at odd s with distinct labels; bwd mirrored
    k = np.zeros((B, S), dtype=np.float32)
    k[:, 3:S:2] = (labels[:, 1:] != labels[:, :-1]).astype(np.float32)
    kL = np.zeros((B, S), dtype=np.float32)
    kL[:, :S - 2] = k[:, 2:]
    kmr = kL[:, ::-1]

    # end-state mask, reversed (bwd init: W = q_255 * em_rev)
    em = np.zeros((B, S), dtype=np.float32)
    rows = np.arange(B)
    em[rows, 2 * lab_len] = 1.0
    em[rows, 2 * lab_len - 1] = 1.0
    emrev = em[:, ::-1]

    # beta-reversal indices for the final dot (shared by all cores):
    # wrapped 16-partition layout, same for both 16-row groups
    i = np.arange(NIDX)
    rvals = np.where(i < S, S - 1 - i, S + 1).astype(np.int16)
    rvw = np.zeros((32, NIDX // 16), dtype=np.int16)
    for g in range(2):
        rvw[16 * g + i % 16, i // 16] = rvals

    in_maps = []
    for c in range(NCORES):
        b0 = BPC * c
        idxw = np.zeros((128, 8 * 12), dtype=np.int16)
        for bg in range(4):
            for g in range(8):
                b = b0 + 8 * bg + g
                idxw[16 * g + i % 16, 12 * (2 * bg + 0) + i // 16] = ext[b, i]
                idxw[16 * g + i % 16, 12 * (2 * bg + 1) + i // 16] = extr[b, i]
        kmc = np.concatenate([k[b0:b0 + BPC], kmr[b0:b0 + BPC]],
                             axis=0).astype(bf)
        emc = np.concatenate([np.zeros((BPC, S), np.float32),
                              emrev[b0:b0 + BPC]], axis=0).astype(bf)
        in_maps.append({
            "yv": yv[b0:b0 + BPC],
            "idxw": idxw,
            "rvw": rvw,
            "km": kmc,
            "emr": emc,
        })
    return in_maps


def _run(in_maps, trace=False):
    from concourse.bass_utils import run_bass_kernel_spmd

    if "nc" not in _cache:
        _cache["nc"] = _build_program()
    return run_bass_kernel_spmd(
        _cache["nc"], in_maps, core_ids=list(range(NCORES)), trace=trace,
    )


def kernel(y_true, y_pred):
    in_maps = _host_prep(y_true, y_pred)
    res = _run(in_maps)
    return np.concatenate([r["loss"] for r in res.results], axis=0)
